# revision 53
# baseline (speedup 1.0000x reference)
"""Multi-head causal attention (B=2, S=2048, D=1024, H=16) on 8 TRN2 NeuronCores.

Sharding: Megatron-style head parallelism. Core c owns heads {2c, 2c+1}:
  - W_q/W_k/W_v column slices [:, 128c:128(c+1)]  (2 heads x 64 dims)
  - attention for those heads over the full sequence (causal)
  - normalized context slices are AllGathered across cores (bf16, 5 chunks
    overlapped with attention of later tiles)
  - each core computes the output-projection column slice TRANSPOSED:
    outT = (ctx_full @ W_o[:, 128c:128(c+1)])^T  as [128, 4096]
  - host concatenates the 8 row blocks and transposes (pure host gather)

Compute dtype: bf16 operands, fp32 PSUM accumulation. Scores are computed
transposed (S^T[k,q] = K Q^T) so the P^T tiles feed the A@V matmul directly;
softmax denominators come from an extra all-ones column appended to V.

v3 structure: x^T and the weight slices are pre-cast to bf16 and
pre-transposed on the host (input layout prep, same as the per-core weight
slicing), so the device does zero casts and zero DMA-transposes — each
512-token x^T chunk is one plain HWDGE load. This matters beyond bytes:
Tile serializes collectives against DMA-transpose instructions, so any
xbar use pushes every AllGather to the end of the kernel. Attention
emission is dovetailed with the x/QKV pipeline so the TensorEngine stays
dense (HAM warm); the scalar queue carries only softmax exps; the gpsimd
queue carries only collectives + post-AG loads. Output projection writes
out^T (no PE transposes; host re-transposes) and m=3 is split per batch so
only the last 512-token slice trails the final AllGather.
"""

import numpy as np
import ml_dtypes

import concourse.bass as bass
import concourse.mybir as mybir
from concourse import bacc, tile
from concourse.masks import make_identity
from concourse.bass_utils import run_bass_kernel_spmd

N_CORES = 8
B, S, D = 2, 2048, 1024
H, DH = 16, 64
BS = B * S  # 4096
HPC = H // N_CORES  # heads per core = 2
DHC = HPC * DH  # 128 context dims per core
SCALE = 1.0 / 32.0  # 1/sqrt(D)
FP32 = mybir.dt.float32
BF16 = mybir.dt.bfloat16
Exp = mybir.ActivationFunctionType.Exp

NQ = 4  # q macro tiles of 512 per batch element
QM = S // NQ  # 512
NKT = S // 128  # 16 k-tiles of 128 per batch element

_nc_cache = {}


def _build():
    nc = bacc.Bacc(
        "TRN2", target_bir_lowering=False, debug=False, num_devices=N_CORES
    )

    xt_d = nc.dram_tensor("xt", [D, BS], BF16, kind="ExternalInput").ap()
    wq_d = nc.dram_tensor("wq", [D, DHC], BF16, kind="ExternalInput").ap()
    wk_d = nc.dram_tensor("wk", [D, DHC], BF16, kind="ExternalInput").ap()
    wv_d = nc.dram_tensor("wv", [D, DHC], BF16, kind="ExternalInput").ap()
    wo_d = nc.dram_tensor("wo", [D, DHC], BF16, kind="ExternalInput").ap()
    tri_d = nc.dram_tensor("tri", [128, 128], BF16, kind="ExternalInput").ap()
    out_d = nc.dram_tensor("out", [DHC, BS], FP32, kind="ExternalOutput").ap()

    with tile.TileContext(nc) as tc:
        with (
            tc.tile_pool(name="dram", bufs=1, space="DRAM") as dram,
            tc.tile_pool(name="pers", bufs=1) as pers,
            tc.tile_pool(name="ptp", bufs=6) as ptp,
            tc.tile_pool(name="nw", bufs=3) as nw,
            tc.tile_pool(name="cfp", bufs=2) as cfp,
            tc.tile_pool(name="ps_s", bufs=2, space="PSUM") as ps_s,
            tc.tile_pool(name="ps_c", bufs=2, space="PSUM") as ps_c,
            tc.tile_pool(name="ps_m", bufs=2, space="PSUM") as ps_m,
        ):
            # ---- persistent SBUF ----
            qt_sb = [pers.tile([128, S], BF16, name=f"qt{b}") for b in range(B)]
            kt_sb = [pers.tile([128, S], BF16, name=f"kt{b}") for b in range(B)]
            # V tiles: per k-tile layout [h0 64 | h1 64] (128 cols)
            v_sb = [pers.tile([128, NKT * 128], BF16, name=f"v{b}") for b in range(B)]
            wq_sb = pers.tile([128, 8, DHC], BF16, name="wq_sb")
            wk_sb = pers.tile([128, 8, DHC], BF16, name="wk_sb")
            wv_sb = pers.tile([128, 8, DHC], BF16, name="wv_sb")
            wo_sb = pers.tile([128, 8, DHC], BF16, name="wo_sb")
            tri_sb = pers.tile([128, 128], BF16, name="tri_sb")
            ones_sb = pers.tile([1, 64], BF16, name="ones_sb")
            id_b = pers.tile([128, 128], BF16, name="id_b")
            xt_sb = pers.tile([128, 8, BS], BF16, name="xt_sb")

            # ---- attention output chunks (m<3: both b; m=3 split per b so
            # the b=0 half AllGathers while b=1's attention still runs) ----
            ctx_in_c = [
                dram.tile([DHC, 2 * QM], BF16, name=f"ctx_in{k}") for k in range(3)
            ] + [
                dram.tile([DHC, QM], BF16, name=f"ctx_in3{s}") for s in "ab"
            ]
            ctx_all_c = [
                dram.tile(
                    [N_CORES * DHC, 2 * QM], BF16, name=f"ctx_all{k}",
                    addr_space="Shared",
                )
                for k in range(3)
            ] + [
                dram.tile(
                    [N_CORES * DHC, QM], BF16, name=f"ctx_all3{s}",
                    addr_space="Shared",
                )
                for s in "ab"
            ]

            # ---- prologue: constants on vector engine, loads on sync ----
            nc.vector.memset(ones_sb[:], 1.0)
            ones128 = pers.tile([128, 1], FP32, name="ones128")
            nc.vector.memset(ones128[:], 1.0)
            make_identity(nc, id_b[:])

            def x_load(b, j):
                """Load one pre-transposed bf16 x^T chunk (plain HWDGE)."""
                g = b * NQ + j
                cols = slice(g * QM, (g + 1) * QM)
                nc.sync.dma_start(
                    xt_sb[:, :, cols],
                    xt_d[:, cols].rearrange("(c p) n -> p c n", p=128),
                )

            # startup: wq first, then the first x chunk in two dt-halves so
            # the dt=0 matmuls of qkv(0,0) can begin mid-load
            nc.sync.dma_start(wq_sb[:], wq_d.rearrange("(c p) n -> p c n", p=128))
            nc.sync.dma_start(
                xt_sb[:, 0:4, 0:QM],
                xt_d[0 : D // 2, 0:QM].rearrange("(c p) n -> p c n", p=128),
            )
            nc.sync.dma_start(wk_sb[:], wk_d.rearrange("(c p) n -> p c n", p=128))
            nc.sync.dma_start(
                xt_sb[:, 4:8, 0:QM],
                xt_d[D // 2 : D, 0:QM].rearrange("(c p) n -> p c n", p=128),
            )
            nc.sync.dma_start(wv_sb[:], wv_d.rearrange("(c p) n -> p c n", p=128))
            x_load(1, 0)
            nc.sync.dma_start(wo_sb[:], wo_d.rearrange("(c p) n -> p c n", p=128))
            nc.sync.dma_start(tri_sb[:], tri_d[:])
            for j in range(1, NQ):
                x_load(0, j)
                x_load(1, j)

            def v_chunk(b, j):
                cols = slice(b * S + j * QM, b * S + (j + 1) * QM)
                vt_ps = ps_m.tile([128, QM], FP32, name="vt_ps", tag="m")
                for dt in range(8):
                    nc.tensor.matmul(
                        vt_ps[:],
                        wv_sb[:, dt, :],
                        xt_sb[:, dt, cols],
                        start=(dt == 0),
                        stop=(dt == 7),
                    )
                vt_sb = nw.tile([128, QM], BF16, name="vt_sb", tag="vt", bufs=2)
                nc.vector.tensor_copy(vt_sb[:], vt_ps[:])
                vtr_ps = ps_m.tile([128, QM], BF16, name="vtr_ps", tag="m")
                for st2 in range(4):
                    nc.tensor.transpose(
                        vtr_ps[:, st2 * 128 : (st2 + 1) * 128],
                        vt_sb[:, st2 * 128 : (st2 + 1) * 128],
                        id_b[:],
                    )
                nc.vector.tensor_copy(
                    v_sb[b][:, j * QM : (j + 1) * QM], vtr_ps[:]
                )

            def qkv_one(b, j):
                cols = slice(b * S + j * QM, b * S + (j + 1) * QM)
                for w_sb, t_sb in ((wq_sb, qt_sb[b]), (wk_sb, kt_sb[b])):
                    ps = ps_m.tile([128, QM], FP32, name="ps_qk", tag="m")
                    for dt in range(8):
                        nc.tensor.matmul(
                            ps[:],
                            w_sb[:, dt, :],
                            xt_sb[:, dt, cols],
                            start=(dt == 0),
                            stop=(dt == 7),
                        )
                    nc.vector.tensor_copy(t_sb[:, j * QM : (j + 1) * QM], ps[:])
                v_chunk(b, j)

            def qkv_pair(j):
                """Both batch elements' chunk j: shares stationary weights."""
                colsb = [
                    slice(b * S + j * QM, b * S + (j + 1) * QM) for b in range(B)
                ]
                for w_sb, dsts in ((wq_sb, qt_sb), (wk_sb, kt_sb)):
                    pss = [
                        ps_m.tile([128, QM], FP32, name=f"ps_qk{b}", tag="m")
                        for b in range(B)
                    ]
                    for dt in range(8):
                        for b in range(B):
                            nc.tensor.matmul(
                                pss[b][:],
                                w_sb[:, dt, :],
                                xt_sb[:, dt, colsb[b]],
                                start=(dt == 0),
                                stop=(dt == 7),
                            )
                    for b in range(B):
                        nc.vector.tensor_copy(
                            dsts[b][:, j * QM : (j + 1) * QM], pss[b][:]
                        )
                for b in range(B):
                    v_chunk(b, j)

            def attention(b, m, pending=None):
                """Emit the kt loop + PSUM eviction; return the norm tail.

                `pending` (the previous tile's tail) is injected two k-tiles
                into this tile's loop: late enough that this tile's first
                scores cover the reciprocal latency, early enough that the
                previous chunk's AllGather input lands ~a full tile sooner."""
                qcols = slice(m * QM, (m + 1) * QM)
                ctx_ps = ps_c.tile([128, QM], FP32, name="ctx_ps", tag="c")
                dacc = nw.tile([128, 2 * QM], FP32, name="dacc", tag="dacc", bufs=2)
                n_kt = 4 * m + 4

                def score_mm(kt):
                    s_ps = ps_s.tile([128, 2 * QM], FP32, name="s_ps", tag="s")
                    for h in range(HPC):
                        nc.tensor.matmul(
                            s_ps[:, h * QM : (h + 1) * QM],
                            kt_sb[b][h * 64 : (h + 1) * 64, kt * 128 : (kt + 1) * 128],
                            qt_sb[b][h * 64 : (h + 1) * 64, qcols],
                            start=True,
                            stop=True,
                            tile_position=(h * 64, 0),
                        )
                    return s_ps

                s_cur = score_mm(0)
                for kt in range(n_kt):
                    s_nxt = score_mm(kt + 1) if kt + 1 < n_kt else None
                    s_ps = s_cur
                    j = kt - 4 * m  # diagonal block index if >= 0
                    qs = max(0, 128 * j)
                    pt = ptp.tile([128, 2 * QM], BF16, name="pt")
                    if j < 0:
                        nc.scalar.activation(pt[:], s_ps[:], Exp, scale=SCALE)
                    else:
                        for h in range(HPC):
                            nc.scalar.activation(
                                pt[:, h * QM + qs : (h + 1) * QM],
                                s_ps[:, h * QM + qs : (h + 1) * QM],
                                Exp,
                                scale=SCALE,
                            )
                            nc.vector.tensor_mul(
                                pt[:, h * QM + qs : h * QM + qs + 128],
                                pt[:, h * QM + qs : h * QM + qs + 128],
                                tri_sb[:],
                            )
                    # softmax denominators: accumulate exp sums on the DVE
                    # (only the freshly-written slice for diagonal tiles)
                    if kt == 0:
                        nc.vector.tensor_copy(dacc[:], pt[:])
                    elif j < 0:
                        nc.vector.tensor_add(dacc[:], dacc[:], pt[:])
                    else:
                        for h in range(HPC):
                            sl = slice(h * QM + qs, (h + 1) * QM)
                            nc.vector.tensor_add(dacc[:, sl], dacc[:, sl], pt[:, sl])
                    # both heads' A@V run concurrently via column tiling
                    for h in range(HPC):
                        nc.tensor.matmul(
                            ctx_ps[h * 64 : (h + 1) * 64, qs:QM],
                            v_sb[b][:, kt * 128 + h * 64 : kt * 128 + (h + 1) * 64],
                            pt[:, h * QM + qs : (h + 1) * QM],
                            start=(kt == 0),
                            stop=(kt == n_kt - 1),
                            tile_position=(0, h * 64),
                        )
                    s_cur = s_nxt
                    if kt == 3 and pending is not None:
                        pending()
                        pending = None
                if pending is not None:
                    pending()
                # evict accumulators to SBUF (frees the PSUM banks); reduce
                # the DVE-accumulated exp sums across partitions with two
                # tiny col-tiled ones-matmuls, then reciprocals as
                # Exp(-Ln(d)) in two scalar table passes over ONE [1, 2*QM]
                # row; the PE-side normalize is deferred into the next tile
                # (kt==3 gives ~4us of cover for the den/Ln/Exp chain).
                ctxa_l = []
                for h in range(HPC):
                    ctxa = nw.tile([64, QM], FP32, name="ctxa", tag="ctxa", bufs=4)
                    nc.vector.tensor_copy(ctxa[:], ctx_ps[h * 64 : (h + 1) * 64, :])
                    ctxa_l.append(ctxa)
                den_ps = ps_m.tile([128, QM], FP32, name="den_ps", tag="m")
                for h in range(HPC):
                    nc.tensor.matmul(
                        den_ps[32 * h : 32 * h + 1, :],
                        ones128[:],
                        dacc[:, h * QM : (h + 1) * QM],
                        start=True,
                        stop=True,
                        tile_position=(0, 32 * h),
                    )
                den = nw.tile([1, 2 * QM], FP32, name="den", tag="den", bufs=2)
                for h in range(HPC):
                    nc.vector.tensor_copy(
                        den[0:1, h * QM : (h + 1) * QM],
                        den_ps[32 * h : 32 * h + 1, :],
                    )
                lnd = nw.tile([1, 2 * QM], FP32, name="lnd", tag="lnd", bufs=2)
                nc.scalar.activation(
                    lnd[:], den[:], mybir.ActivationFunctionType.Ln
                )
                recip = nw.tile([1, 2 * QM], BF16, name="recip", tag="recip", bufs=2)
                nc.scalar.activation(recip[:], lnd[:], Exp, scale=-1.0)
                recip_l = [recip[0:1, h * QM : (h + 1) * QM] for h in range(HPC)]

                def tail():
                    for h in range(HPC):
                        bc_ps = ps_m.tile([128, QM], FP32, name="bc_ps", tag="m")
                        nc.tensor.matmul(
                            bc_ps[0:64, :], ones_sb[:], recip_l[h],
                            start=True, stop=True,
                        )
                        ctxn = nw.tile([64, QM], BF16, name="ctxn")
                        nc.vector.tensor_mul(
                            ctxn[:], ctxa_l[h][:], bc_ps[0:64, :]
                        )
                        if m < 3:
                            dst_ap = ctx_in_c[m][
                                h * 64 : (h + 1) * 64, b * QM : (b + 1) * QM
                            ]
                        else:
                            dst_ap = ctx_in_c[3 + b][h * 64 : (h + 1) * 64, :]
                        nc.sync.dma_start(dst_ap, ctxn[:])

                return tail

            def allgather(k):
                nc.gpsimd.collective_compute(
                    "AllGather",
                    mybir.AluOpType.bypass,
                    replica_groups=[list(range(N_CORES))],
                    ins=[ctx_in_c[k][:]],
                    outs=[ctx_all_c[k][:]],
                )

            def cf_load(k):
                """Pull the AllGathered chunk k into SBUF (gpsimd queue:
                naturally ordered after the AG completion wait)."""
                wide = 2 * QM if k < 3 else QM
                tag = "cf" if k < 3 else "cf3"
                cf = cfp.tile([128, 8, wide], BF16, name=f"cf{k}", tag=tag, bufs=2)
                nc.gpsimd.dma_start(
                    cf[:], ctx_all_c[k][:].rearrange("(c p) n -> p c n", p=128)
                )
                return cf

            def outproj_mm(m, cf, bbs):
                """outT[:, tokens] = sum_dt W_o[dt-block]^T @ ctxT[dt-block];
                batch halves share each stationary weight load when paired."""
                ots = [
                    ps_m.tile([128, QM], FP32, name=f"ot_ps{i}", tag="m")
                    for i in range(len(bbs))
                ]
                for dt in range(8):
                    for i in range(len(bbs)):
                        nc.tensor.matmul(
                            ots[i][:],
                            wo_sb[:, dt, :],
                            cf[:, dt, i * QM : (i + 1) * QM],
                            start=(dt == 0),
                            stop=(dt == 7),
                        )
                for i, bb in enumerate(bbs):
                    ot_sb = nw.tile([128, QM], FP32, name="ot_sb", tag="ot", bufs=2)
                    if bb % 2 == 0:
                        nc.scalar.copy(ot_sb[:], ots[i][:])
                    else:
                        nc.vector.tensor_copy(ot_sb[:], ots[i][:])
                    nc.sync.dma_start(
                        out_d[:, bb * S + m * QM : bb * S + (m + 1) * QM],
                        ot_sb[:],
                    )

            # ---- main emission: dovetail qkv / attention / AG / outproj
            qkv_one(0, 0)
            t00 = attention(0, 0)
            qkv_one(1, 0)
            t10 = attention(1, 0, pending=t00)
            qkv_pair(1)
            t01 = attention(0, 1, pending=t10)
            allgather(0)
            t11 = attention(1, 1, pending=t01)
            qkv_pair(2)
            t02 = attention(0, 2, pending=t11)
            allgather(1)
            cf0 = cf_load(0)
            t12 = attention(1, 2, pending=t02)
            qkv_pair(3)
            t03 = attention(0, 3, pending=t12)
            allgather(2)
            cf1 = cf_load(1)
            outproj_mm(0, cf0, (0, 1))
            t13 = attention(1, 3, pending=t03)
            allgather(3)
            t13()
            allgather(4)
            cf2 = cf_load(2)
            outproj_mm(1, cf1, (0, 1))
            cf3a = cf_load(3)
            outproj_mm(2, cf2, (0, 1))
            outproj_mm(3, cf3a, (0,))
            cf3b = cf_load(4)
            outproj_mm(3, cf3b, (1,))

    nc.compile()
    return nc


def _build_nc():
    if "nc" not in _nc_cache:
        _nc_cache["nc"] = _build()
    return _nc_cache["nc"]


def kernel(x, W_q, W_k, W_v, W_o):
    BF = ml_dtypes.bfloat16
    x = np.asarray(x, dtype=np.float32).reshape(BS, D)
    xt = np.ascontiguousarray(x.T).astype(BF)  # [D, BS] bf16
    # keep-mask for the diagonal 128x128 block of S^T[k, q]: keep k <= q
    tri = np.triu(np.ones((128, 128), dtype=np.float32)).astype(BF)
    in_maps = []
    for c in range(N_CORES):
        sl = slice(c * DHC, (c + 1) * DHC)
        in_maps.append(
            {
                "xt": xt,
                "wq": np.ascontiguousarray(np.asarray(W_q, np.float32)[:, sl]).astype(BF),
                "wk": np.ascontiguousarray(np.asarray(W_k, np.float32)[:, sl]).astype(BF),
                "wv": np.ascontiguousarray(np.asarray(W_v, np.float32)[:, sl]).astype(BF),
                "wo": np.ascontiguousarray(np.asarray(W_o, np.float32)[:, sl]).astype(BF),
                "tri": tri,
            }
        )
    nc = _build_nc()
    res = run_bass_kernel_spmd(nc, in_maps, core_ids=list(range(N_CORES)))
    outT = np.concatenate(
        [res.results[c]["out"] for c in range(N_CORES)], axis=0
    )  # [D, BS]
    return np.ascontiguousarray(outT.T).reshape(B, S, D)


# revision 54
# speedup vs baseline: 1.0449x; 1.0449x over previous
"""Multi-head causal attention (B=2, S=2048, D=1024, H=16) on 8 TRN2 NeuronCores.

Sharding: Megatron-style head parallelism. Core c owns heads {2c, 2c+1}:
  - W_q/W_k/W_v column slices [:, 128c:128(c+1)]  (2 heads x 64 dims)
  - attention for those heads over the full sequence (causal)
  - normalized context slices are AllGathered across cores (bf16, 5 chunks
    overlapped with attention of later tiles)
  - each core computes the output-projection column slice TRANSPOSED:
    outT = (ctx_full @ W_o[:, 128c:128(c+1)])^T  as [128, 4096]
  - host concatenates the 8 row blocks and transposes (pure host gather)

Compute dtype: bf16 operands, fp32 PSUM accumulation. Scores are computed
transposed (S^T[k,q] = K Q^T) so the P^T tiles feed the A@V matmul directly;
softmax denominators come from an extra all-ones column appended to V.

v3 structure: x^T and the weight slices are pre-cast to bf16 and
pre-transposed on the host (input layout prep, same as the per-core weight
slicing), so the device does zero casts and zero DMA-transposes — each
512-token x^T chunk is one plain HWDGE load. This matters beyond bytes:
Tile serializes collectives against DMA-transpose instructions, so any
xbar use pushes every AllGather to the end of the kernel. Attention
emission is dovetailed with the x/QKV pipeline so the TensorEngine stays
dense (HAM warm); the scalar queue carries only softmax exps; the gpsimd
queue carries only collectives + post-AG loads. Output projection writes
out^T (no PE transposes; host re-transposes) and m=3 is split per batch so
only the last 512-token slice trails the final AllGather.
"""

import numpy as np
import ml_dtypes

import concourse.bass as bass
import concourse.mybir as mybir
from concourse import bacc, tile
from concourse.masks import make_identity
from concourse.bass_utils import run_bass_kernel_spmd

N_CORES = 8
B, S, D = 2, 2048, 1024
H, DH = 16, 64
BS = B * S  # 4096
HPC = H // N_CORES  # heads per core = 2
DHC = HPC * DH  # 128 context dims per core
SCALE = 1.0 / 32.0  # 1/sqrt(D)
FP32 = mybir.dt.float32
BF16 = mybir.dt.bfloat16
Exp = mybir.ActivationFunctionType.Exp

NQ = 4  # q macro tiles of 512 per batch element
QM = S // NQ  # 512
NKT = S // 128  # 16 k-tiles of 128 per batch element

_nc_cache = {}


def _build():
    nc = bacc.Bacc(
        "TRN2", target_bir_lowering=False, debug=False, num_devices=N_CORES
    )

    xt_d = nc.dram_tensor("xt", [D, BS], BF16, kind="ExternalInput").ap()
    wq_d = nc.dram_tensor("wq", [D, DHC], BF16, kind="ExternalInput").ap()
    wk_d = nc.dram_tensor("wk", [D, DHC], BF16, kind="ExternalInput").ap()
    wv_d = nc.dram_tensor("wv", [D, DHC], BF16, kind="ExternalInput").ap()
    wo_d = nc.dram_tensor("wo", [D, DHC], BF16, kind="ExternalInput").ap()
    tri_d = nc.dram_tensor("tri", [128, 128], BF16, kind="ExternalInput").ap()
    out_d = nc.dram_tensor("out", [DHC, BS], FP32, kind="ExternalOutput").ap()

    with tile.TileContext(nc) as tc:
        with (
            tc.tile_pool(name="dram", bufs=1, space="DRAM") as dram,
            tc.tile_pool(name="pers", bufs=1) as pers,
            tc.tile_pool(name="ptp", bufs=6) as ptp,
            tc.tile_pool(name="nw", bufs=3) as nw,
            tc.tile_pool(name="cfp", bufs=2) as cfp,
            tc.tile_pool(name="ps_s", bufs=2, space="PSUM") as ps_s,
            tc.tile_pool(name="ps_c", bufs=2, space="PSUM") as ps_c,
            tc.tile_pool(name="ps_m", bufs=2, space="PSUM") as ps_m,
        ):
            # ---- persistent SBUF ----
            qt_sb = [pers.tile([128, S], BF16, name=f"qt{b}") for b in range(B)]
            kt_sb = [pers.tile([128, S], BF16, name=f"kt{b}") for b in range(B)]
            # V tiles: per k-tile layout [h0 64 | ones | h1 64 | ones] (130 cols)
            v_sb = [pers.tile([128, NKT * 130], BF16, name=f"v{b}") for b in range(B)]
            wq_sb = pers.tile([128, 8, DHC], BF16, name="wq_sb")
            wk_sb = pers.tile([128, 8, DHC], BF16, name="wk_sb")
            wv_sb = pers.tile([128, 8, DHC], BF16, name="wv_sb")
            wo_sb = pers.tile([128, 8, DHC], BF16, name="wo_sb")
            tri_sb = pers.tile([128, 128], BF16, name="tri_sb")
            ones_sb = pers.tile([1, 64], BF16, name="ones_sb")
            id_b = pers.tile([128, 128], BF16, name="id_b")
            xt_sb = pers.tile([128, 8, BS], BF16, name="xt_sb")

            # ---- attention output chunks (m<3: both b; m=3 split per b so
            # the b=0 half AllGathers while b=1's attention still runs) ----
            ctx_in_c = [
                dram.tile([DHC, 2 * QM], BF16, name=f"ctx_in{k}") for k in range(3)
            ] + [
                dram.tile([DHC, QM], BF16, name=f"ctx_in3{s}") for s in "ab"
            ]
            ctx_all_c = [
                dram.tile(
                    [N_CORES * DHC, 2 * QM], BF16, name=f"ctx_all{k}",
                    addr_space="Shared",
                )
                for k in range(3)
            ] + [
                dram.tile(
                    [N_CORES * DHC, QM], BF16, name=f"ctx_all3{s}",
                    addr_space="Shared",
                )
                for s in "ab"
            ]

            # ---- prologue: constants on vector engine, loads on sync ----
            nc.vector.memset(ones_sb[:], 1.0)
            make_identity(nc, id_b[:])
            for b in range(B):
                nc.vector.memset(v_sb[b][:], 1.0)

            def x_load(b, j):
                """Load one pre-transposed bf16 x^T chunk (plain HWDGE)."""
                g = b * NQ + j
                cols = slice(g * QM, (g + 1) * QM)
                nc.sync.dma_start(
                    xt_sb[:, :, cols],
                    xt_d[:, cols].rearrange("(c p) n -> p c n", p=128),
                )

            # startup: wq first, then the first x chunk in two dt-halves so
            # the dt=0 matmuls of qkv(0,0) can begin mid-load
            nc.sync.dma_start(wq_sb[:], wq_d.rearrange("(c p) n -> p c n", p=128))
            nc.sync.dma_start(
                xt_sb[:, 0:4, 0:QM],
                xt_d[0 : D // 2, 0:QM].rearrange("(c p) n -> p c n", p=128),
            )
            nc.sync.dma_start(wk_sb[:], wk_d.rearrange("(c p) n -> p c n", p=128))
            nc.sync.dma_start(
                xt_sb[:, 4:8, 0:QM],
                xt_d[D // 2 : D, 0:QM].rearrange("(c p) n -> p c n", p=128),
            )
            nc.sync.dma_start(wv_sb[:], wv_d.rearrange("(c p) n -> p c n", p=128))
            x_load(1, 0)
            nc.sync.dma_start(wo_sb[:], wo_d.rearrange("(c p) n -> p c n", p=128))
            nc.sync.dma_start(tri_sb[:], tri_d[:])
            for j in range(1, NQ):
                x_load(0, j)
                x_load(1, j)

            def v_chunk(b, j):
                cols = slice(b * S + j * QM, b * S + (j + 1) * QM)
                vt_ps = ps_m.tile([128, QM], FP32, name="vt_ps", tag="m")
                for dt in range(8):
                    nc.tensor.matmul(
                        vt_ps[:],
                        wv_sb[:, dt, :],
                        xt_sb[:, dt, cols],
                        start=(dt == 0),
                        stop=(dt == 7),
                    )
                vt_sb = nw.tile([128, QM], BF16, name="vt_sb", tag="vt", bufs=2)
                nc.vector.tensor_copy(vt_sb[:], vt_ps[:])
                vtr_ps = ps_m.tile([128, QM], BF16, name="vtr_ps", tag="m")
                for st2 in range(4):
                    nc.tensor.transpose(
                        vtr_ps[:, st2 * 128 : (st2 + 1) * 128],
                        vt_sb[:, st2 * 128 : (st2 + 1) * 128],
                        id_b[:],
                    )
                dst = v_sb[b][:, j * 520 : (j + 1) * 520].rearrange(
                    "p (t g c) -> p t g c", t=4, g=2
                )[:, :, :, 0:64]
                vsrc = vtr_ps[:].rearrange("p (t g c) -> p t g c", t=4, g=2)
                nc.vector.tensor_copy(dst, vsrc)

            def qkv_one(b, j):
                cols = slice(b * S + j * QM, b * S + (j + 1) * QM)
                for w_sb, t_sb in ((wq_sb, qt_sb[b]), (wk_sb, kt_sb[b])):
                    ps = ps_m.tile([128, QM], FP32, name="ps_qk", tag="m")
                    for dt in range(8):
                        nc.tensor.matmul(
                            ps[:],
                            w_sb[:, dt, :],
                            xt_sb[:, dt, cols],
                            start=(dt == 0),
                            stop=(dt == 7),
                        )
                    nc.vector.tensor_copy(t_sb[:, j * QM : (j + 1) * QM], ps[:])
                v_chunk(b, j)

            def qkv_pair(j):
                """Both batch elements' chunk j: shares stationary weights."""
                colsb = [
                    slice(b * S + j * QM, b * S + (j + 1) * QM) for b in range(B)
                ]
                for w_sb, dsts in ((wq_sb, qt_sb), (wk_sb, kt_sb)):
                    pss = [
                        ps_m.tile([128, QM], FP32, name=f"ps_qk{b}", tag="m")
                        for b in range(B)
                    ]
                    for dt in range(8):
                        for b in range(B):
                            nc.tensor.matmul(
                                pss[b][:],
                                w_sb[:, dt, :],
                                xt_sb[:, dt, colsb[b]],
                                start=(dt == 0),
                                stop=(dt == 7),
                            )
                    for b in range(B):
                        nc.vector.tensor_copy(
                            dsts[b][:, j * QM : (j + 1) * QM], pss[b][:]
                        )
                for b in range(B):
                    v_chunk(b, j)

            def attention(b, m, pending=None):
                """Emit the kt loop + PSUM eviction; return the norm tail.

                `pending` (the previous tile's tail) is injected two k-tiles
                into this tile's loop: late enough that this tile's first
                scores cover the reciprocal latency, early enough that the
                previous chunk's AllGather input lands ~a full tile sooner."""
                qcols = slice(m * QM, (m + 1) * QM)
                ctx_ps = [
                    ps_c.tile([65, QM], FP32, name=f"ctx_ps{h}", tag="c")
                    for h in range(HPC)
                ]
                n_kt = 4 * m + 4

                def score_mm(kt):
                    s_ps = ps_s.tile([128, 2 * QM], FP32, name="s_ps", tag="s")
                    for h in range(HPC):
                        nc.tensor.matmul(
                            s_ps[:, h * QM : (h + 1) * QM],
                            kt_sb[b][h * 64 : (h + 1) * 64, kt * 128 : (kt + 1) * 128],
                            qt_sb[b][h * 64 : (h + 1) * 64, qcols],
                            start=True,
                            stop=True,
                            tile_position=(h * 64, 0),
                        )
                    return s_ps

                s_cur = score_mm(0)
                for kt in range(n_kt):
                    s_nxt = score_mm(kt + 1) if kt + 1 < n_kt else None
                    s_ps = s_cur
                    j = kt - 4 * m  # diagonal block index if >= 0
                    qs = max(0, 128 * j)
                    pt = ptp.tile([128, 2 * QM], BF16, name="pt")
                    if j < 0:
                        nc.scalar.activation(pt[:], s_ps[:], Exp, scale=SCALE)
                    else:
                        for h in range(HPC):
                            nc.scalar.activation(
                                pt[:, h * QM + qs : (h + 1) * QM],
                                s_ps[:, h * QM + qs : (h + 1) * QM],
                                Exp,
                                scale=SCALE,
                            )
                            nc.vector.tensor_mul(
                                pt[:, h * QM + qs : h * QM + qs + 128],
                                pt[:, h * QM + qs : h * QM + qs + 128],
                                tri_sb[:],
                            )
                    for h in range(HPC):
                        nc.tensor.matmul(
                            ctx_ps[h][:, qs:QM],
                            v_sb[b][:, kt * 130 + h * 65 : kt * 130 + (h + 1) * 65],
                            pt[:, h * QM + qs : (h + 1) * QM],
                            start=(kt == 0),
                            stop=(kt == n_kt - 1),
                        )
                    s_cur = s_nxt
                    if kt == 3 and pending is not None:
                        pending()
                        pending = None
                if pending is not None:
                    pending()
                # evict accumulators to SBUF (frees the PSUM banks) and
                # compute both heads' reciprocals as Exp(-Ln(d)) in two
                # scalar-engine table passes over ONE [1, 2*QM] row (~1.8us
                # total vs 6.6us for the single-lane DVE reciprocal); the
                # PE-side normalize is deferred into the next tile (kt==3
                # gives ~4us of cover for the den-copy + Ln + Exp chain).
                ctxa_l = []
                den = nw.tile([1, 2 * QM], FP32, name="den", tag="den", bufs=2)
                for h in range(HPC):
                    ctxa = nw.tile([64, QM], FP32, name="ctxa", tag="ctxa", bufs=4)
                    nc.vector.tensor_copy(ctxa[:], ctx_ps[h][0:64, :])
                    nc.vector.tensor_copy(
                        den[0:1, h * QM : (h + 1) * QM], ctx_ps[h][64:65, :]
                    )
                    ctxa_l.append(ctxa)
                lnd = nw.tile([1, 2 * QM], FP32, name="lnd", tag="lnd", bufs=2)
                nc.scalar.activation(
                    lnd[:], den[:], mybir.ActivationFunctionType.Ln
                )
                recip = nw.tile([1, 2 * QM], BF16, name="recip", tag="recip", bufs=2)
                nc.scalar.activation(recip[:], lnd[:], Exp, scale=-1.0)
                recip_l = [recip[0:1, h * QM : (h + 1) * QM] for h in range(HPC)]

                def tail():
                    for h in range(HPC):
                        bc_ps = ps_m.tile([128, QM], FP32, name="bc_ps", tag="m")
                        nc.tensor.matmul(
                            bc_ps[0:64, :], ones_sb[:], recip_l[h],
                            start=True, stop=True,
                        )
                        ctxn = nw.tile([64, QM], BF16, name="ctxn")
                        nc.vector.tensor_mul(
                            ctxn[:], ctxa_l[h][:], bc_ps[0:64, :]
                        )
                        if m < 3:
                            dst_ap = ctx_in_c[m][
                                h * 64 : (h + 1) * 64, b * QM : (b + 1) * QM
                            ]
                        else:
                            dst_ap = ctx_in_c[3 + b][h * 64 : (h + 1) * 64, :]
                        nc.sync.dma_start(dst_ap, ctxn[:])

                return tail

            def allgather(k):
                nc.gpsimd.collective_compute(
                    "AllGather",
                    mybir.AluOpType.bypass,
                    replica_groups=[list(range(N_CORES))],
                    ins=[ctx_in_c[k][:]],
                    outs=[ctx_all_c[k][:]],
                )

            def cf_load(k):
                """Pull the AllGathered chunk k into SBUF (gpsimd queue:
                naturally ordered after the AG completion wait)."""
                wide = 2 * QM if k < 3 else QM
                tag = "cf" if k < 3 else "cf3"
                cf = cfp.tile([128, 8, wide], BF16, name=f"cf{k}", tag=tag, bufs=2)
                nc.gpsimd.dma_start(
                    cf[:], ctx_all_c[k][:].rearrange("(c p) n -> p c n", p=128)
                )
                return cf

            def outproj_mm(m, cf, bbs):
                """outT[:, tokens] = sum_dt W_o[dt-block]^T @ ctxT[dt-block];
                batch halves share each stationary weight load when paired."""
                ots = [
                    ps_m.tile([128, QM], FP32, name=f"ot_ps{i}", tag="m")
                    for i in range(len(bbs))
                ]
                for dt in range(8):
                    for i in range(len(bbs)):
                        nc.tensor.matmul(
                            ots[i][:],
                            wo_sb[:, dt, :],
                            cf[:, dt, i * QM : (i + 1) * QM],
                            start=(dt == 0),
                            stop=(dt == 7),
                        )
                for i, bb in enumerate(bbs):
                    ot_sb = nw.tile([128, QM], FP32, name="ot_sb", tag="ot", bufs=2)
                    if bb % 2 == 0:
                        nc.scalar.copy(ot_sb[:], ots[i][:])
                    else:
                        nc.vector.tensor_copy(ot_sb[:], ots[i][:])
                    nc.sync.dma_start(
                        out_d[:, bb * S + m * QM : bb * S + (m + 1) * QM],
                        ot_sb[:],
                    )

            # ---- main emission: dovetail qkv / attention / AG / outproj
            qkv_one(0, 0)
            t00 = attention(0, 0)
            qkv_one(1, 0)
            t10 = attention(1, 0, pending=t00)
            qkv_pair(1)
            t01 = attention(0, 1, pending=t10)
            allgather(0)
            t11 = attention(1, 1, pending=t01)
            qkv_pair(2)
            t02 = attention(0, 2, pending=t11)
            allgather(1)
            cf0 = cf_load(0)
            t12 = attention(1, 2, pending=t02)
            qkv_pair(3)
            t03 = attention(0, 3, pending=t12)
            allgather(2)
            cf1 = cf_load(1)
            outproj_mm(0, cf0, (0, 1))
            t13 = attention(1, 3, pending=t03)
            allgather(3)
            t13()
            allgather(4)
            cf2 = cf_load(2)
            outproj_mm(1, cf1, (0, 1))
            cf3a = cf_load(3)
            outproj_mm(2, cf2, (0, 1))
            outproj_mm(3, cf3a, (0,))
            cf3b = cf_load(4)
            outproj_mm(3, cf3b, (1,))

    nc.compile()
    return nc


def _build_nc():
    if "nc" not in _nc_cache:
        _nc_cache["nc"] = _build()
    return _nc_cache["nc"]


def kernel(x, W_q, W_k, W_v, W_o):
    BF = ml_dtypes.bfloat16
    x = np.asarray(x, dtype=np.float32).reshape(BS, D)
    xt = np.ascontiguousarray(x.T).astype(BF)  # [D, BS] bf16
    # keep-mask for the diagonal 128x128 block of S^T[k, q]: keep k <= q
    tri = np.triu(np.ones((128, 128), dtype=np.float32)).astype(BF)
    in_maps = []
    for c in range(N_CORES):
        sl = slice(c * DHC, (c + 1) * DHC)
        in_maps.append(
            {
                "xt": xt,
                "wq": np.ascontiguousarray(np.asarray(W_q, np.float32)[:, sl]).astype(BF),
                "wk": np.ascontiguousarray(np.asarray(W_k, np.float32)[:, sl]).astype(BF),
                "wv": np.ascontiguousarray(np.asarray(W_v, np.float32)[:, sl]).astype(BF),
                "wo": np.ascontiguousarray(np.asarray(W_o, np.float32)[:, sl]).astype(BF),
                "tri": tri,
            }
        )
    nc = _build_nc()
    res = run_bass_kernel_spmd(nc, in_maps, core_ids=list(range(N_CORES)))
    outT = np.concatenate(
        [res.results[c]["out"] for c in range(N_CORES)], axis=0
    )  # [D, BS]
    return np.ascontiguousarray(outT.T).reshape(B, S, D)
